# revision 9
# baseline (speedup 1.0000x reference)
"""AlphaCompositor on 8 TRN2 NeuronCores.

Data-parallel over the view axis N (one image per core). Per core:
  - prefetch the K=16 fragment planes and pre-clamp the gather indices,
  - per-pixel exclusive cumprod of (1-alpha) on DVE,
  - gather rgba point features from the host-transposed (P,4) table with
    the SWDGE indirect row-gather (the HW instruction gathers one table
    row per destination partition, i.e. 128 indices/call -> 512 calls
    per plane),
  - weighted accumulate on DVE, background fill, write out.
"""

import sys

sys.path.insert(0, "/opt/trn_rl_repo")

import numpy as np

N, K, H, W = 8, 16, 256, 256
C, P = 4, 100000
PIX = H * W  # 65536
PPART = 128
FREE = PIX // PPART  # 512

_CACHE = {}


def _build_nc():
    import concourse.bass as bass
    import concourse.mybir as mybir
    import concourse.tile as tile
    from concourse import bacc

    f32 = mybir.dt.float32
    i32 = mybir.dt.int32
    Alu = mybir.AluOpType

    nc = bacc.Bacc(None, target_bir_lowering=False)
    frag_d = nc.declare_dram_parameter("frag", [K, PIX], i32, isOutput=False)
    alpha_d = nc.declare_dram_parameter("alpha", [K, PIX], f32, isOutput=False)
    table_d = nc.declare_dram_parameter("table", [P, C], f32, isOutput=False)
    bg_d = nc.declare_dram_parameter("bg", [1, C], f32, isOutput=False)
    out_d = nc.declare_dram_parameter("out", [C, PIX], f32, isOutput=True)

    with tile.TileContext(nc) as tc:
        with (
            tc.tile_pool(name="io", bufs=6) as io_pool,
            tc.tile_pool(name="g", bufs=4) as g_pool,
            tc.tile_pool(name="persist", bufs=1) as pp,
        ):
            acc = pp.tile([PPART, FREE, C], f32)
            t = pp.tile([PPART, FREE], f32)
            bg = pp.tile([PPART, 1, C], f32)
            nc.vector.memset(t[:], 1.0)
            nc.sync.dma_start(out=bg[:, 0, :], in_=bg_d[:, :].to_broadcast([PPART, C]))

            for k in range(K):
                fk = io_pool.tile([PPART, FREE], i32, tag="frag")
                ak = io_pool.tile([PPART, FREE], f32, tag="alpha")
                idx = io_pool.tile([PPART, FREE], i32, tag="idx")
                nc.sync.dma_start(
                    out=fk[:], in_=frag_d[k].rearrange("(p f) -> p f", p=PPART)
                )
                nc.sync.dma_start(
                    out=ak[:], in_=alpha_d[k].rearrange("(p f) -> p f", p=PPART)
                )
                nc.vector.tensor_scalar_max(idx[:], fk[:], 0)

                # gather: HW indirect DMA is a per-partition row gather
                # (128 indices/call, one per partition) -> 512 calls/plane
                Gt = g_pool.tile([PPART, FREE * C], f32, tag="G")
                for w in range(FREE):
                    nc.gpsimd.indirect_dma_start(
                        out=Gt[:, w * C : (w + 1) * C],
                        out_offset=None,
                        in_=table_d[:],
                        in_offset=bass.IndirectOffsetOnAxis(
                            ap=idx[:, w : w + 1], axis=0
                        ),
                    )
                G3 = Gt[:].rearrange("p (f c) -> p f c", c=C)

                # a = (frag >= 0) * alpha
                a = io_pool.tile([PPART, FREE], f32, tag="a")
                nc.vector.scalar_tensor_tensor(
                    out=a[:], in0=fk[:], scalar=0, in1=ak[:],
                    op0=Alu.is_ge, op1=Alu.mult,
                )
                # w = a * t ; t -= w
                w = io_pool.tile([PPART, FREE], f32, tag="w")
                nc.vector.tensor_tensor(out=w[:], in0=a[:], in1=t[:], op=Alu.mult)
                if k == 0:
                    # background mask from the nearest fragment plane:
                    # acc starts at m * bg (m==1 -> all weights are 0)
                    m = io_pool.tile([PPART, FREE], f32, tag="m")
                    nc.vector.tensor_scalar(
                        out=m[:], in0=fk[:], scalar1=0, scalar2=None, op0=Alu.is_lt
                    )
                    m3 = m[:].rearrange("p (f o) -> p f o", o=1).to_broadcast(
                        [PPART, FREE, C]
                    )
                    bg3 = bg[:].to_broadcast([PPART, FREE, C])
                    nc.vector.tensor_tensor(out=acc[:], in0=m3, in1=bg3, op=Alu.mult)
                if k < K - 1:
                    nc.vector.tensor_tensor(out=t[:], in0=t[:], in1=w[:], op=Alu.subtract)

                # G *= w (broadcast over rgba) ; acc += G
                w3 = w[:].rearrange("p (f o) -> p f o", o=1).to_broadcast(
                    [PPART, FREE, C]
                )
                nc.vector.tensor_tensor(out=G3, in0=G3, in1=w3, op=Alu.mult)
                nc.vector.tensor_tensor(out=acc[:], in0=acc[:], in1=G3, op=Alu.add)

            # unpack interleaved rgba planes and store
            for c in range(C):
                pl = io_pool.tile([PPART, FREE], f32, tag="pl")
                nc.scalar.copy(out=pl[:], in_=acc[:, :, c])
                nc.sync.dma_start(
                    out=out_d[c].rearrange("(p f) -> p f", p=PPART), in_=pl[:]
                )

    nc.compile()
    return nc


def _get_nc():
    if "nc" not in _CACHE:
        _CACHE["nc"] = _build_nc()
    return _CACHE["nc"]


def _run(fragments, alphas, ptclds, background_color, trace=False, **kw):
    from concourse.bass_utils import run_bass_kernel_spmd

    nc = _get_nc()

    table = np.ascontiguousarray(ptclds.T).astype(np.float32)  # (P, C)
    bg4 = np.concatenate(
        [background_color.astype(np.float32), np.ones(1, np.float32)]
    ).reshape(1, C)

    in_maps = []
    for i in range(N):
        in_maps.append(
            {
                "frag": np.ascontiguousarray(fragments[i].reshape(K, PIX)),
                "alpha": np.ascontiguousarray(alphas[i].reshape(K, PIX)),
                "table": table,
                "bg": bg4,
            }
        )

    res = run_bass_kernel_spmd(nc, in_maps, core_ids=list(range(N)), trace=trace, **kw)
    out = np.stack([res.results[i]["out"].reshape(C, H, W) for i in range(N)])
    return out.astype(np.float32), res


def kernel(fragments, alphas, ptclds, background_color):
    out, _ = _run(fragments, alphas, ptclds, background_color)
    return out


# revision 10
# speedup vs baseline: 1.1705x; 1.1705x over previous
"""AlphaCompositor on 8 TRN2 NeuronCores.

Data-parallel over the view axis N (one image per core). Per core:
  - prefetch the K=16 fragment planes and pre-clamp the gather indices,
  - per-pixel exclusive cumprod of (1-alpha) on DVE,
  - gather rgba point features from the host-transposed (P,4) table with
    the SWDGE indirect row-gather (the HW instruction gathers one table
    row per destination partition, i.e. 128 indices/call -> 512 calls
    per plane),
  - weighted accumulate on DVE, background fill, write out.
"""

import sys

sys.path.insert(0, "/opt/trn_rl_repo")

import numpy as np

N, K, H, W = 8, 16, 256, 256
C, P = 4, 100000
PIX = H * W  # 65536
PPART = 128
FREE = PIX // PPART  # 512

_CACHE = {}


def _build_nc():
    import concourse.bass as bass
    import concourse.mybir as mybir
    import concourse.tile as tile
    from concourse import bacc

    f32 = mybir.dt.float32
    i32 = mybir.dt.int32
    Alu = mybir.AluOpType

    nc = bacc.Bacc(None, target_bir_lowering=False)
    frag_d = nc.declare_dram_parameter("frag", [K, PIX], i32, isOutput=False)
    alpha_d = nc.declare_dram_parameter("alpha", [K, PIX], f32, isOutput=False)
    table_d = nc.declare_dram_parameter("table", [P, C], f32, isOutput=False)
    bg_d = nc.declare_dram_parameter("bg", [1, C], f32, isOutput=False)
    out_d = nc.declare_dram_parameter("out", [C, PIX], f32, isOutput=True)

    with tile.TileContext(nc) as tc:
        with (
            tc.tile_pool(name="io", bufs=4) as io_pool,
            tc.tile_pool(name="g", bufs=4) as g_pool,
            tc.tile_pool(name="persist", bufs=1) as pp,
        ):
            acc = pp.tile([PPART, FREE, C], f32)
            t = pp.tile([PPART, FREE], f32)
            bg = pp.tile([PPART, 1, C], f32)
            nc.vector.memset(t[:], 1.0)
            nc.sync.dma_start(out=bg[:, 0, :], in_=bg_d[:, :].to_broadcast([PPART, C]))

            for k in range(K):
                fk = io_pool.tile([PPART, FREE], i32, tag="frag")
                ak = io_pool.tile([PPART, FREE], f32, tag="alpha")
                nc.sync.dma_start(
                    out=fk[:], in_=frag_d[k].rearrange("(p f) -> p f", p=PPART)
                )
                nc.sync.dma_start(
                    out=ak[:], in_=alpha_d[k].rearrange("(p f) -> p f", p=PPART)
                )

                # gather: HW indirect DMA is a per-partition row gather
                # (128 indices/call, one per partition) -> 512 calls/plane
                Gt = g_pool.tile([PPART, FREE * C], f32, tag="G")
                idx = io_pool.tile([PPART, FREE], i32, tag="idx")
                nc.vector.tensor_scalar_max(idx[:], fk[:], 0)
                for w in range(FREE):
                    nc.gpsimd.indirect_dma_start(
                        out=Gt[:, w * C : (w + 1) * C],
                        out_offset=None,
                        in_=table_d[:],
                        in_offset=bass.IndirectOffsetOnAxis(
                            ap=idx[:, w : w + 1], axis=0
                        ),
                    )
                G3 = Gt[:].rearrange("p (f c) -> p f c", c=C)

                # a = (frag >= 0) * alpha
                a = io_pool.tile([PPART, FREE], f32, tag="a")
                nc.vector.scalar_tensor_tensor(
                    out=a[:], in0=fk[:], scalar=0, in1=ak[:],
                    op0=Alu.is_ge, op1=Alu.mult,
                )
                # w = a * t ; t -= w
                w = io_pool.tile([PPART, FREE], f32, tag="w")
                nc.vector.tensor_tensor(out=w[:], in0=a[:], in1=t[:], op=Alu.mult)
                if k == 0:
                    # background mask from the nearest fragment plane:
                    # acc starts at m * bg (m==1 -> all weights are 0)
                    m = io_pool.tile([PPART, FREE], f32, tag="m")
                    nc.vector.tensor_scalar(
                        out=m[:], in0=fk[:], scalar1=0, scalar2=None, op0=Alu.is_lt
                    )
                    m3 = m[:].rearrange("p (f o) -> p f o", o=1).to_broadcast(
                        [PPART, FREE, C]
                    )
                    bg3 = bg[:].to_broadcast([PPART, FREE, C])
                    nc.vector.tensor_tensor(out=acc[:], in0=m3, in1=bg3, op=Alu.mult)
                if k < K - 1:
                    nc.vector.tensor_tensor(out=t[:], in0=t[:], in1=w[:], op=Alu.subtract)

                # G *= w (broadcast over rgba) ; acc += G
                w3 = w[:].rearrange("p (f o) -> p f o", o=1).to_broadcast(
                    [PPART, FREE, C]
                )
                nc.vector.tensor_tensor(out=G3, in0=G3, in1=w3, op=Alu.mult)
                nc.vector.tensor_tensor(out=acc[:], in0=acc[:], in1=G3, op=Alu.add)

            # unpack interleaved rgba planes and store
            for c in range(C):
                pl = io_pool.tile([PPART, FREE], f32, tag="pl")
                nc.scalar.copy(out=pl[:], in_=acc[:, :, c])
                nc.sync.dma_start(
                    out=out_d[c].rearrange("(p f) -> p f", p=PPART), in_=pl[:]
                )

    nc.compile()
    return nc


def _get_nc():
    if "nc" not in _CACHE:
        _CACHE["nc"] = _build_nc()
    return _CACHE["nc"]


def _run(fragments, alphas, ptclds, background_color, trace=False, **kw):
    from concourse.bass_utils import run_bass_kernel_spmd

    nc = _get_nc()

    table = np.ascontiguousarray(ptclds.T).astype(np.float32)  # (P, C)
    bg4 = np.concatenate(
        [background_color.astype(np.float32), np.ones(1, np.float32)]
    ).reshape(1, C)

    in_maps = []
    for i in range(N):
        in_maps.append(
            {
                "frag": np.ascontiguousarray(fragments[i].reshape(K, PIX)),
                "alpha": np.ascontiguousarray(alphas[i].reshape(K, PIX)),
                "table": table,
                "bg": bg4,
            }
        )

    res = run_bass_kernel_spmd(nc, in_maps, core_ids=list(range(N)), trace=trace, **kw)
    out = np.stack([res.results[i]["out"].reshape(C, H, W) for i in range(N)])
    return out.astype(np.float32), res


def kernel(fragments, alphas, ptclds, background_color):
    out, _ = _run(fragments, alphas, ptclds, background_color)
    return out
